# revision 22
# baseline (speedup 1.0000x reference)
"""Trainium2 Bass kernel for nn_DecoderRNN (LSTM + Bahdanau attention + vocab FC).

Sharding: data-parallel over batch (B=64 -> 8 per core) for attention+LSTM;
tensor-parallel over vocab for the FC (AllGather of the 1.3MB h-history, then
each core computes all 1280 (b,t) rows x its 3840-col vocab slice).

Key structure (vs reference):
  - emb projection (emb @ W_ih[:E] + b_ih + b_hh) computed host-side (exact fp32).
  - Z = feat_flat @ W_ih[E:] precomputed once on device; the per-step
    context+input-projection collapses to gatesT += Z.T @ A where A[j, b] =
    alpha[b, p] * [j == b*49+p] (block-diagonal), built from a constant mask.
  - b_att dropped (softmax shift-invariant); b_enc+b_dec folded into enc_projT.
  - Per step the Whh-part matmuls are emitted BEFORE the attention chain so the
    tensor engine stays busy while vector/scalar compute softmax.
  - FC logits written bf16; b_fc added host-side.
"""
import numpy as np

B, T, P, F, E, H, V = 64, 20, 49, 2048, 256, 512, 30000
NC = 8            # cores
BC = B // NC      # 8 batches per core
J = BC * P        # 392 flattened (b, p) rows per core
G4 = 4 * H        # 2048 gate width
VP = 30720        # V padded to NC * 3840
VS = VP // NC     # 3840 vocab cols per core (tensor-parallel FC)
JT = [128, 128, 128, J - 384]   # j k-tile sizes
HT = H // 128     # 4 h k-tiles
FT = F // 128     # 16 f k-tiles
GMT = G4 // 128   # 16 gate m-tiles
TB = T * BC       # 160 (t,b) rows per core
RALL = B * T      # 1280 global rows for FC
NRT = RALL // 128  # 10 row tiles
NVC = VS // 480    # 8 psum chunks of 480 cols

_cache = {}


def _build_program():
    import concourse.bacc as bacc
    import concourse.mybir as mybir
    import concourse.tile as tile

    dt = mybir.dt
    AF = mybir.ActivationFunctionType
    ALU = mybir.AluOpType

    nc = bacc.Bacc("TRN2", target_bir_lowering=False, debug=False, num_devices=NC)

    def din(name, shape, dtype):
        return nc.dram_tensor(name, shape, dtype, kind="ExternalInput").ap()

    featT_d = din("featT", [F, J], dt.bfloat16)        # feat[f, b*49+p]
    wenc_d = din("wenc", [F, H], dt.bfloat16)
    wic_d = din("wic", [F, G4], dt.bfloat16)
    wdec_d = din("wdec", [H, H], dt.bfloat16)
    whh_d = din("whh", [H, G4], dt.bfloat16)
    vatt_d = din("vatt", [H, 1], dt.bfloat16)
    biasT_d = din("biasT", [H, 1], dt.float32)         # b_enc + b_dec
    embpT_d = din("embpT", [128, GMT * T * BC], dt.float32)  # [g_lo,(mt,t,b)]
    mask_d = din("mask", [J, BC], dt.bfloat16)         # block-diag indicator
    maskT_d = din("maskT", [BC, J], dt.bfloat16)
    mask1_d = din("mask1", [128, 32], dt.bfloat16)
    ones_d = din("ones11", [1, 1], dt.float32)
    wfc_d = din("wfc", [H, VS], dt.bfloat16)           # per-core vocab slice

    out_d = nc.dram_tensor("logits", [RALL, VS], dt.bfloat16,
                           kind="ExternalOutput").ap()

    with tile.TileContext(nc, num_cores=NC) as tc:
        with (
            tc.tile_pool(name="const", bufs=1) as cpool,
            tc.tile_pool(name="persist", bufs=1) as pp,
            tc.tile_pool(name="work", bufs=2) as wk,
            tc.tile_pool(name="dram", bufs=1, space="DRAM") as dram,
        ):
            # ---- consolidated constant loads (few big DMAs) ----
            p0cm = tc.tile_pool(name="p0pool", bufs=1)
            p0pool = p0cm.__enter__()
            featT = p0pool.tile([128, FT * J], dt.bfloat16, tag="featT", name="featT")
            wenc = p0pool.tile([128, FT * H], dt.bfloat16, tag="wenc", name="wenc")
            wic = p0pool.tile([128, FT * G4], dt.bfloat16, tag="wic", name="wic")
            wdec = cpool.tile([128, HT * H], dt.bfloat16, tag="wdec", name="wdec")
            whh = cpool.tile([128, HT * G4], dt.bfloat16, tag="whh", name="whh")
            vatt = cpool.tile([128, HT], dt.bfloat16, tag="vatt", name="vatt")
            biasT = cpool.tile([128, HT], dt.float32, tag="biasT", name="biasT")
            mask = [cpool.tile([JT[k], BC], dt.bfloat16, tag=f"mask{k}", name=f"mask{k}") for k in range(4)]
            maskT = cpool.tile([BC, J], dt.bfloat16, tag="maskT", name="maskT")
            ones11 = cpool.tile([1, 1], dt.float32, tag="ones11", name="ones11")

            ft3 = featT[:].rearrange("p (k j) -> p k j", k=FT)
            we3 = wenc[:].rearrange("p (k h) -> p k h", k=FT)
            wi3 = wic[:].rearrange("p (k g) -> p k g", k=FT)
            wd3 = wdec[:].rearrange("p (k h) -> p k h", k=HT)
            wh3 = whh[:].rearrange("p (k g) -> p k g", k=HT)

            ftd = featT_d.rearrange("(k p) j -> p k j", p=128)
            wed = wenc_d.rearrange("(k p) h -> p k h", p=128)
            wid = wic_d.rearrange("(k p) g -> p k g", p=128)
            nc.gpsimd.dma_start(ft3[:, 0:8], ftd[:, 0:8])
            nc.sync.dma_start(ft3[:, 8:16], ftd[:, 8:16])
            nc.gpsimd.dma_start(we3[:, 0:8], wed[:, 0:8])
            nc.sync.dma_start(we3[:, 8:16], wed[:, 8:16])
            nc.scalar.dma_start(wi3[:, 0:4], wid[:, 0:4])
            nc.gpsimd.dma_start(wi3[:, 4:8], wid[:, 4:8])
            nc.sync.dma_start(wi3[:, 8:12], wid[:, 8:12])
            nc.scalar.dma_start(wi3[:, 12:16], wid[:, 12:16])
            nc.scalar.dma_start(wd3, wdec_d.rearrange("(k p) h -> p k h", p=128))
            nc.scalar.dma_start(wh3, whh_d.rearrange("(k p) g -> p k g", p=128))
            nc.gpsimd.dma_start(vatt[:], vatt_d.rearrange("(k p) o -> p (k o)", p=128))
            nc.gpsimd.dma_start(biasT[:], biasT_d.rearrange("(k p) o -> p (k o)", p=128))
            off = 0
            for k in range(4):
                nc.gpsimd.dma_start(mask[k][:], mask_d[off:off + JT[k], :])
                off += JT[k]
            nc.gpsimd.dma_start(maskT[:], maskT_d[:])
            nc.gpsimd.dma_start(ones11[:], ones_d[:])

            ps0cm = tc.tile_pool(name="ps0", bufs=2, space="PSUM")
            ps0 = ps0cm.__enter__()

            # ---- P0a: enc_projT[h, j] = sum_f wenc[f, h] * featT[f, j] + bias
            epT = pp.tile([128, HT * J], dt.bfloat16, tag="epT", name="epT")
            for m in range(HT):
                acc = ps0.tile([128, 512], dt.float32, tag="p0", name="ps_ep")[:, :J]
                for k in range(FT):
                    nc.tensor.matmul(acc[:], we3[:, k, m * 128:(m + 1) * 128],
                                     ft3[:, k, :], start=(k == 0), stop=(k == FT - 1))
                nc.vector.tensor_scalar_add(epT[:, m * J:(m + 1) * J], acc[:],
                                            biasT[:, m:m + 1])

            # ---- P0b: Z[j, g] = sum_f featT[f, j] * wic[f, g]
            Zt = [pp.tile([JT[k], G4], dt.bfloat16, tag=f"Zt{k}", name=f"Zt{k}") for k in range(4)]
            off = 0
            for jm in range(4):
                js = JT[jm]
                for nch in range(4):
                    acc = ps0.tile([128, 512], dt.float32, tag="p0", name="ps_z")[:js, :]
                    for k in range(FT):
                        nc.tensor.matmul(
                            acc[:], ft3[:, k, off:off + js],
                            wi3[:, k, nch * 512:(nch + 1) * 512],
                            start=(k == 0), stop=(k == FT - 1))
                    nc.vector.tensor_copy(Zt[jm][:, nch * 512:(nch + 1) * 512], acc[:])
                off += js

            ps0cm.__exit__(None, None, None)
            p0cm.__exit__(None, None, None)

            # wfc + gathered-h buffers: allocated after the P0 weights free up;
            # the 3.9MB wfc DMA overlaps the recurrence
            latecm = tc.tile_pool(name="late", bufs=1)
            latep = latecm.__enter__()
            mask1 = latep.tile([128, 32], dt.bfloat16, tag="mask1", name="mask1")
            nc.gpsimd.dma_start(mask1[:], mask1_d[:])
            embpT = latep.tile([128, GMT * T * BC], dt.float32, tag="embpT",
                               name="embpT")
            nc.gpsimd.dma_start(embpT[:], embpT_d[:])
            wfcs = latep.tile([128, HT * VS], dt.bfloat16, tag="wfcs", name="wfcs")
            wf3 = wfcs[:].rearrange("p (k v) -> p k v", k=HT)
            nc.gpsimd.dma_start(wf3, wfc_d.rearrange("(k p) v -> p k v", p=128))
            HcAll = latep.tile([128, HT * RALL], dt.bfloat16, tag="HcAll",
                               name="HcAll")

            # ---- recurrence state ----
            cL = pp.tile([128, HT * BC], dt.float32, tag="cL", name="cL")
            Hc = pp.tile([128, T * HT * BC], dt.bfloat16, tag="Hc", name="Hc")  # [h_lo,(t,h_hi,b)]
            nc.gpsimd.memset(cL[:], 0.0)

            Hc4 = Hc[:].rearrange("p (t h b) -> p t h b", t=T, h=HT)

            def hsl(tt, k):   # h(tt) k-tile [128, BC] inside Hc
                return Hc4[:, tt, k, :]

            hc_inA = dram.tile([128, 16 * HT * BC], dt.bfloat16)
            hc_outA = dram.tile([NC * 128, 16 * HT * BC], dt.bfloat16)
            hc_inB = dram.tile([128, 4 * HT * BC], dt.bfloat16)
            hc_outB = dram.tile([NC * 128, 4 * HT * BC], dt.bfloat16)
            # HcAll rows ordered (c, t, b); host reassembles accordingly
            HcA5A = HcAll[:].rearrange("p (k r) -> p k r", k=HT)

            class _HcA5:
                def __getitem__(self, idx):
                    _, k, _, ts, _ = idx
                    if ts == slice(0, 16):
                        return HcA5A[:, k, 0:1024].rearrange(
                            "p (c t b) -> p c t b", c=NC, t=16)
                    assert ts == slice(16, 20)
                    return HcA5A[:, k, 1024:1280].rearrange(
                        "p (c t b) -> p c t b", c=NC, t=4)
            HcA5 = _HcA5()
            HcStage = latep.tile([128, NC * T * HT * BC], dt.bfloat16,
                                 tag="HcStage", name="HcStage")

            pstcm = tc.tile_pool(name="pst", bufs=1, space="PSUM")
            pst = pstcm.__enter__()
            psgcm = tc.tile_pool(name="psg", bufs=1, space="PSUM")
            psg = psgcm.__enter__()

            for t in range(T):
                # 1. decT[h', b] (skip at t=0: h == 0)
                if t > 0:
                    pdec = pst.tile([128, HT * BC], dt.float32, tag="ps_dec", name="ps_dec")
                    for m in range(HT):
                        for k in range(HT):
                            nc.tensor.matmul(
                                pdec[:, m * BC:(m + 1) * BC],
                                wd3[:, k, m * 128:(m + 1) * 128],
                                hsl(t - 1, k),
                                start=(k == 0), stop=(k == HT - 1))
                # Whh-part into its own psum (complete groups) to keep the
                # tensor engine busy during the softmax chain
                if t > 0:
                    pgh = psg.tile([128, GMT * BC], dt.float32, tag="ps_gh",
                                   name="ps_gh")
                    for m in range(GMT):
                        o = m * BC
                        for k in range(HT):
                            nc.tensor.matmul(pgh[:, o:o + BC],
                                             wh3[:, k, m * 128:(m + 1) * 128],
                                             hsl(t - 1, k),
                                             start=(k == 0), stop=(k == HT - 1))
                # 2. R = relu(epT + decT) ; e = v.T @ R
                pe = pst.tile([1, J], dt.float32, tag="ps_e", name="ps_e")
                R = wk.tile([128, HT * J], dt.bfloat16, tag="R", name="R")
                if t > 0:
                    decS = wk.tile([128, HT * BC], dt.bfloat16, tag="decS",
                                   name="decS")
                    nc.vector.tensor_copy(decS[:], pdec[:])
                    radd = wk.tile([128, HT * J], dt.bfloat16, tag="radd",
                                   name="radd")
                    nc.vector.tensor_tensor(
                        radd[:].rearrange("p (m b q) -> p m b q", m=HT, b=BC),
                        epT[:].rearrange("p (m b q) -> p m b q", m=HT, b=BC),
                        decS[:].rearrange("p (m b) -> p m b", m=HT).unsqueeze(3)
                            .broadcast_to([128, HT, BC, P]),
                        ALU.add)
                    nc.vector.tensor_scalar_max(R[:], radd[:], 0.0)
                else:
                    nc.vector.tensor_scalar_max(R[:], epT[:], 0.0)
                for m in range(HT):
                    nc.tensor.matmul(pe[:], vatt[:, m:m + 1], R[:, m * J:(m + 1) * J],
                                     start=(m == 0), stop=(m == HT - 1))
                # 3. softmax over p within each b
                # 3b. s = sigmoid(-e) row; transpose to columns (exact fp32)
                sgn = wk.tile([1, J], dt.float32, tag="sgn", name="sgn")
                nc.scalar.activation(sgn[:], pe[:], AF.Sigmoid, scale=-1.0)
                pa = pst.tile([128, 12], dt.float32, tag="ps_a", name="ps_a")
                off = 0
                for k in range(4):
                    nc.tensor.matmul(pa[:JT[k], k:k + 1],
                                     sgn[:, off:off + JT[k]], ones11[:],
                                     start=True, stop=True)
                    off += JT[k]
                # exp(e) = 1/s - 1, per-column land (128 lanes)
                exr = wk.tile([128, 4], dt.float32, tag="exr", name="exr")
                nc.vector.reciprocal(exr[:], pa[:, 0:4])
                exmb = wk.tile([128, 4], dt.bfloat16, tag="exmb", name="exmb")
                nc.vector.tensor_scalar_add(exmb[:], exr[:], -1.0)
                # per-b sums via mask.T matmul -> pa[0:8, 8]
                for k in range(4):
                    nc.tensor.matmul(pa[0:BC, 8:9], mask[k][:],
                                     exmb[:JT[k], k:k + 1],
                                     start=(k == 0), stop=(k == 3))
                rsc = wk.tile([BC, 1], dt.float32, tag="rsc", name="rsc")
                nc.vector.reciprocal(rsc[:], pa[0:BC, 8:9])
                rscb = wk.tile([BC, 1], dt.bfloat16, tag="rscb", name="rscb")
                nc.vector.tensor_copy(rscb[:], rsc[:])
                # rs392[j] = rs[j//49] via maskT matmul -> pa[:, 4+k]
                off = 0
                for k in range(4):
                    nc.tensor.matmul(pa[:JT[k], 4 + k:5 + k],
                                     maskT[:, off:off + JT[k]], rscb[:],
                                     start=True, stop=True)
                    off += JT[k]
                # 4. A1 = mask1 * exp_col * rs392 (merged [128,(kt,b)], 2 ops)
                At = wk.tile([128, 32], dt.bfloat16, tag="At", name="At")
                nc.vector.tensor_tensor(
                    At[:].rearrange("p (k b) -> p k b", k=4),
                    mask1[:].rearrange("p (k b) -> p k b", k=4),
                    exmb[:].unsqueeze(2).broadcast_to([128, 4, BC]),
                    ALU.mult)
                nc.vector.tensor_tensor(
                    At[:].rearrange("p (k b) -> p k b", k=4),
                    At[:].rearrange("p (k b) -> p k b", k=4),
                    pa[:, 4:8].unsqueeze(2).broadcast_to([128, 4, BC]),
                    ALU.mult)
                # 5. gatesT[g, b] = Z.T @ A
                pg = psg.tile([128, GMT * BC], dt.float32, tag="ps_g", name="ps_g")
                for m in range(GMT):
                    o = m * BC
                    for k in range(4):
                        nc.tensor.matmul(pg[:, o:o + BC],
                                         Zt[k][:, m * 128:(m + 1) * 128],
                                         At[:JT[k], k * BC:(k + 1) * BC],
                                         start=(k == 0),
                                         stop=(k == 3))
                # 6. gates + embp (+ Whh-part), LSTM elementwise.
                # Split per h-tile k so h[k] lands early and the next step's
                # dec/Whh matmuls (waiting per k-slice) restart sooner.
                gL = wk.tile([128, GMT * BC], dt.float32, tag="gL", name="gL")
                gL4 = gL[:].rearrange("p (q k b) -> p q k b", q=4, k=HT)
                pg4 = pg[:].rearrange("p (q k b) -> p q k b", q=4, k=HT)
                em5 = embpT[:].rearrange("p (q k t b) -> p q k t b", q=4,
                                         k=HT, t=T)
                cL4 = cL[:].rearrange("p (k b) -> p k b", k=HT)
                for k in range(HT):
                    nc.vector.tensor_tensor(gL4[:, :, k], pg4[:, :, k],
                                            em5[:, :, k, t], ALU.add)
                    if t > 0:
                        pgh4 = pgh[:].rearrange("p (q k b) -> p q k b", q=4,
                                                k=HT)
                        nc.vector.tensor_tensor(gL4[:, :, k], gL4[:, :, k],
                                                pgh4[:, :, k], ALU.add)
                    sg = wk.tile([128, 3 * BC], dt.float32, tag=f"sg{k}",
                                 name=f"sg{k}")
                    sg3 = sg[:].rearrange("p (q b) -> p q b", q=3)
                    nc.scalar.activation(sg3, gL4[:, 0:3, k], AF.Sigmoid)
                    tg = wk.tile([128, BC], dt.float32, tag=f"tg{k}",
                                 name=f"tg{k}")
                    nc.scalar.activation(tg[:], gL4[:, 3, k], AF.Tanh)
                    si, sf, so = (sg[:, 0:BC], sg[:, BC:2 * BC],
                                  sg[:, 2 * BC:3 * BC])
                    t1 = wk.tile([128, BC], dt.float32, tag=f"t1{k}",
                                 name=f"t1{k}")
                    nc.vector.tensor_tensor(t1[:], sf, cL4[:, k], ALU.mult)
                    t2 = wk.tile([128, BC], dt.float32, tag=f"t2{k}",
                                 name=f"t2{k}")
                    nc.vector.tensor_tensor(t2[:], si, tg[:], ALU.mult)
                    nc.vector.tensor_tensor(cL4[:, k], t1[:], t2[:], ALU.add)
                    th = wk.tile([128, BC], dt.float32, tag=f"th{k}",
                                 name=f"th{k}")
                    nc.scalar.activation(th[:], cL4[:, k], AF.Tanh)
                    nc.vector.tensor_tensor(Hc4[:, t, k, :], so, th[:],
                                            ALU.mult)
                if t == 15:
                    # gather steps 0..15 now; latency hides under steps 16..19
                    nc.gpsimd.dma_start(hc_inA[:], Hc[:, 0:16 * HT * BC])
                    nc.gpsimd.collective_compute(
                        "AllGather", mybir.AluOpType.bypass,
                        replica_groups=[list(range(NC))],
                        ins=[hc_inA.opt()], outs=[hc_outA.opt()])

            psgcm.__exit__(None, None, None)
            pstcm.__exit__(None, None, None)

            # ---- tail AllGather (steps 16..19); hides under FC on t<16 rows
            nc.gpsimd.dma_start(hc_inB[:], Hc[:, 16 * HT * BC:])
            nc.gpsimd.collective_compute(
                "AllGather", mybir.AluOpType.bypass,
                replica_groups=[list(range(NC))],
                ins=[hc_inB.opt()], outs=[hc_outB.opt()])
            # load-backs on the scalar queue (don't block gpsimd), then
            # on-chip vector reshuffle into HcAll
            hoA = hc_outA[:].rearrange("(c p) r -> p c r", p=128)
            hoB = hc_outB[:].rearrange("(c p) r -> p c r", p=128)
            for c in range(NC):
                nc.scalar.dma_start(
                    HcStage[:, c * 512:(c + 1) * 512], hoA[:, c])
            hsA = HcStage[:, 0:NC * 512].rearrange(
                "p (c t k b) -> p c t k b", c=NC, t=16, k=HT)
            for k in range(HT):
                nc.vector.tensor_copy(HcA5[:, k, :, 0:16, :],
                                      hsA[:, :, :, k, :])
            for c in range(NC):
                nc.scalar.dma_start(
                    HcStage[:, NC * 512 + c * 128:NC * 512 + (c + 1) * 128],
                    hoB[:, c])
            hsB = HcStage[:, NC * 512:].rearrange(
                "p (c t k b) -> p c t k b", c=NC, t=4, k=HT)
            for k in range(HT):
                nc.vector.tensor_copy(HcA5[:, k, :, 16:20, :],
                                      hsB[:, :, :, k, :])

            # ---- FC: out[r, v] = sum_h HcAll[h, r] * wfc[h, v], rows r=(b,t)
            psfcm = tc.tile_pool(name="psfc", bufs=1, space="PSUM")
            psf = psfcm.__enter__()
            focm = tc.tile_pool(name="fcout", bufs=2)
            fco = focm.__enter__()
            for rt in range(NRT):
                pls = [psf.tile([128, 480], dt.float32, tag=f"ps_l{n}",
                                name=f"ps_l{n}") for n in range(NVC)]
                for k in range(HT):
                    st = HcAll[:, k * RALL + rt * 128:k * RALL + (rt + 1) * 128]
                    for n in range(NVC):
                        nc.tensor.matmul(pls[n][:], st,
                                         wf3[:, k, n * 480:(n + 1) * 480],
                                         start=(k == 0), stop=(k == HT - 1))
                outrow = fco.tile([128, VS], dt.bfloat16, tag="outrow",
                                  name="outrow")
                for n in range(NVC):
                    nc.vector.tensor_copy(outrow[:, n * 480:(n + 1) * 480],
                                          pls[n][:])
                nc.gpsimd.dma_start(out_d[rt * 128:(rt + 1) * 128, :], outrow[:])
            focm.__exit__(None, None, None)
            psfcm.__exit__(None, None, None)
            latecm.__exit__(None, None, None)
    nc.compile()
    return nc


def _prep_inputs(features, captions, emb_table, W_enc, b_enc, W_dec, b_dec,
                 v_att, b_att, W_ih, b_ih, W_hh, b_hh, W_fc, b_fc):
    f32 = np.float32
    import ml_dtypes
    bf16 = ml_dtypes.bfloat16

    # gate permutation (i, f, g, o) -> (i, f, o, g) on the 4H axis
    gperm = np.concatenate([np.arange(0, H), np.arange(H, 2 * H),
                            np.arange(3 * H, 4 * H), np.arange(2 * H, 3 * H)])
    emb = np.asarray(emb_table, f32)[np.asarray(captions)]        # [B,T,E]
    embp = emb.reshape(B * T, E) @ np.asarray(W_ih, f32)[:E]      # [B*T,4H]
    embp += (np.asarray(b_ih, f32) + np.asarray(b_hh, f32))
    embp = embp.reshape(B, T, G4)[:, :, gperm]

    wicT = np.ascontiguousarray(np.asarray(W_ih, f32)[E:][:, gperm]).astype(bf16)
    wencT = np.asarray(W_enc, f32).astype(bf16)                   # [F,H]
    wdecT = np.asarray(W_dec, f32).astype(bf16)                   # [H,H]
    whhT = np.ascontiguousarray(np.asarray(W_hh, f32)[:, gperm]).astype(bf16)
    vattc = np.asarray(v_att, f32).reshape(H, 1).astype(bf16)
    biasT = (np.asarray(b_enc, f32) + np.asarray(b_dec, f32)).reshape(H, 1)
    wfc = np.zeros((H, VP), f32)
    wfc[:, :V] = np.asarray(W_fc, f32)
    wfc = wfc.astype(bf16)

    maskM = np.zeros((J, BC), f32)
    for j in range(J):
        maskM[j, j // P] = 1.0
    maskM = maskM.astype(bf16)
    maskT = np.ascontiguousarray(maskM.T)
    mask1 = np.zeros((128, 32), f32)
    for k in range(4):
        js = [128, 128, 128, J - 384][k]
        for r in range(js):
            j = k * 128 + r
            mask1[r, k * 8 + j // P] = 1.0
    mask1 = mask1.astype(bf16)
    ones11 = np.ones((1, 1), f32)

    feats = np.asarray(features, f32)
    in_maps = []
    for c in range(NC):
        fs = feats[c * BC:(c + 1) * BC]                           # [8,49,F]
        featT = np.ascontiguousarray(
            fs.reshape(J, F).T).astype(bf16)                      # [F, J]
        ep = embp[c * BC:(c + 1) * BC]                            # [8,T,4H]
        # embpT[g_lo, (mt, t, b)] = ep[b, t, mt*128+g_lo]
        epr = ep.transpose(2, 1, 0).reshape(GMT, 128, T, BC)      # [mt,g_lo,t,b]
        embpT = np.ascontiguousarray(
            epr.transpose(1, 0, 2, 3).reshape(128, GMT * T * BC))
        in_maps.append({
            "featT": featT, "wenc": wencT, "wic": wicT, "wdec": wdecT,
            "whh": whhT, "vatt": vattc, "biasT": biasT.astype(f32),
            "embpT": embpT.astype(f32), "mask": maskM, "maskT": maskT,
            "mask1": mask1, "ones11": ones11,
            "wfc": np.ascontiguousarray(wfc[:, c * VS:(c + 1) * VS]),
        })
    return in_maps


def _install_ntff_hook_shim():
    """Synthesize antenv.axon_hooks (missing in this image) so
    run_bass_kernel_spmd(trace=True) can NTFF-profile via libaxon."""
    import sys, types, ctypes, contextlib
    try:
        from antenv.axon_hooks import get_axon_ntff_profile_hook  # noqa
        return
    except ImportError:
        pass
    so_path = "/opt/axon/libaxon_pjrt.so"
    lib = ctypes.CDLL(so_path)
    lib.axon_start_nrt_profile.argtypes = [ctypes.POINTER(ctypes.c_int64),
                                           ctypes.c_size_t]
    lib.axon_start_nrt_profile.restype = ctypes.c_int64
    lib.axon_stop_nrt_profile.argtypes = [ctypes.c_char_p]
    lib.axon_stop_nrt_profile.restype = ctypes.c_int64

    @contextlib.contextmanager
    def _hook(output_dir, device_ids):
        import jax
        jax.devices()
        if device_ids:
            ids = (ctypes.c_int64 * len(device_ids))(*device_ids)
            rc = lib.axon_start_nrt_profile(ids, len(device_ids))
        else:
            rc = lib.axon_start_nrt_profile(None, 0)
        if rc != 0:
            raise RuntimeError(f"axon_start_nrt_profile rc={rc}")
        try:
            yield
        finally:
            n = lib.axon_stop_nrt_profile(str(output_dir).encode())
            print(f"profile: {n} file(s) written to {output_dir}",
                  file=sys.stderr)

    mod = types.ModuleType("antenv.axon_hooks")
    mod.get_axon_ntff_profile_hook = lambda: _hook
    mod.set_axon_ntff_profile_hook = lambda h: None
    sys.modules["antenv.axon_hooks"] = mod


def kernel(**inputs):
    import os
    from concourse.bass_utils import run_bass_kernel_spmd
    if "nc" not in _cache:
        _cache["nc"] = _build_program()
    nc = _cache["nc"]
    in_maps = _prep_inputs(**inputs)
    trace = bool(int(os.environ.get("KERNEL_TRACE", "0")))
    if trace:
        _install_ntff_hook_shim()
    res = run_bass_kernel_spmd(nc, in_maps, list(range(NC)), trace=trace,
                               tmpdir=os.environ.get("KERNEL_TRACE_DIR"))
    _cache["last_res"] = res
    # each core returns all 1280 (b,t) rows x its 3840-col vocab slice
    def reorder(arr):
        a = arr[:1024].reshape(NC, 16, BC, VS).transpose(0, 2, 1, 3)
        b = arr[1024:].reshape(NC, 4, BC, VS).transpose(0, 2, 1, 3)
        return np.concatenate([a, b], axis=2).reshape(B, T, VS)
    parts = [reorder(np.asarray(res.results[c]["logits"]).astype(np.float32))
             for c in range(NC)]
    out = np.concatenate(parts, axis=2)[:, :, :V]
    out += np.asarray(inputs["b_fc"], np.float32)[None, None, :]
    return out


# revision 23
# speedup vs baseline: 1.0774x; 1.0774x over previous
"""Trainium2 Bass kernel for nn_DecoderRNN (LSTM + Bahdanau attention + vocab FC).

Sharding: data-parallel over batch (B=64 -> 8 per core) for attention+LSTM;
tensor-parallel over vocab for the FC (AllGather of the 1.3MB h-history, then
each core computes all 1280 (b,t) rows x its 3840-col vocab slice).

Key structure (vs reference):
  - emb projection (emb @ W_ih[:E] + b_ih + b_hh) computed host-side (exact fp32).
  - Z = feat_flat @ W_ih[E:] precomputed once on device; the per-step
    context+input-projection collapses to gatesT += Z.T @ A where A[j, b] =
    alpha[b, p] * [j == b*49+p] (block-diagonal), built from a constant mask.
  - b_att dropped (softmax shift-invariant); b_enc+b_dec folded into enc_projT.
  - Per step the Whh-part matmuls are emitted BEFORE the attention chain so the
    tensor engine stays busy while vector/scalar compute softmax.
  - FC logits written bf16; b_fc added host-side.
"""
import numpy as np

B, T, P, F, E, H, V = 64, 20, 49, 2048, 256, 512, 30000
NC = 8            # cores
BC = B // NC      # 8 batches per core
J = BC * P        # 392 flattened (b, p) rows per core
G4 = 4 * H        # 2048 gate width
VP = 30720        # V padded to NC * 3840
VS = VP // NC     # 3840 vocab cols per core (tensor-parallel FC)
JT = [128, 128, 128, J - 384]   # j k-tile sizes
HT = H // 128     # 4 h k-tiles
FT = F // 128     # 16 f k-tiles
GMT = G4 // 128   # 16 gate m-tiles
TB = T * BC       # 160 (t,b) rows per core
RALL = B * T      # 1280 global rows for FC
NRT = RALL // 128  # 10 row tiles
NVC = VS // 480    # 8 psum chunks of 480 cols

_cache = {}


def _build_program():
    import concourse.bacc as bacc
    import concourse.mybir as mybir
    import concourse.tile as tile

    dt = mybir.dt
    AF = mybir.ActivationFunctionType
    ALU = mybir.AluOpType

    nc = bacc.Bacc("TRN2", target_bir_lowering=False, debug=False, num_devices=NC)

    def din(name, shape, dtype):
        return nc.dram_tensor(name, shape, dtype, kind="ExternalInput").ap()

    featT_d = din("featT", [F, J], dt.bfloat16)        # feat[f, b*49+p]
    wenc_d = din("wenc", [F, H], dt.bfloat16)
    wic_d = din("wic", [F, G4], dt.bfloat16)
    wdec_d = din("wdec", [H, H], dt.bfloat16)
    whh_d = din("whh", [H, G4], dt.bfloat16)
    vatt_d = din("vatt", [H, 1], dt.bfloat16)
    biasT_d = din("biasT", [H, 1], dt.float32)         # b_enc + b_dec
    embpT_d = din("embpT", [128, GMT * T * BC], dt.float32)  # [g_lo,(mt,t,b)]
    mask_d = din("mask", [J, BC], dt.bfloat16)         # block-diag indicator
    maskT_d = din("maskT", [BC, J], dt.bfloat16)
    mask1_d = din("mask1", [128, 32], dt.bfloat16)
    ones_d = din("ones11", [1, 1], dt.float32)
    wfc_d = din("wfc", [H, VS], dt.bfloat16)           # per-core vocab slice

    out_d = nc.dram_tensor("logits", [RALL, VS], dt.bfloat16,
                           kind="ExternalOutput").ap()

    with tile.TileContext(nc, num_cores=NC) as tc:
        with (
            tc.tile_pool(name="const", bufs=1) as cpool,
            tc.tile_pool(name="persist", bufs=1) as pp,
            tc.tile_pool(name="work", bufs=2) as wk,
            tc.tile_pool(name="dram", bufs=1, space="DRAM") as dram,
        ):
            # ---- consolidated constant loads (few big DMAs) ----
            p0cm = tc.tile_pool(name="p0pool", bufs=1)
            p0pool = p0cm.__enter__()
            featT = p0pool.tile([128, FT * J], dt.bfloat16, tag="featT", name="featT")
            wenc = p0pool.tile([128, FT * H], dt.bfloat16, tag="wenc", name="wenc")
            wic = p0pool.tile([128, FT * G4], dt.bfloat16, tag="wic", name="wic")
            wdec = cpool.tile([128, HT * H], dt.bfloat16, tag="wdec", name="wdec")
            whh = cpool.tile([128, HT * G4], dt.bfloat16, tag="whh", name="whh")
            vatt = cpool.tile([128, HT], dt.bfloat16, tag="vatt", name="vatt")
            biasT = cpool.tile([128, HT], dt.float32, tag="biasT", name="biasT")
            mask = [cpool.tile([JT[k], BC], dt.bfloat16, tag=f"mask{k}", name=f"mask{k}") for k in range(4)]
            maskT = cpool.tile([BC, J], dt.bfloat16, tag="maskT", name="maskT")
            ones11 = cpool.tile([1, 1], dt.float32, tag="ones11", name="ones11")

            ft3 = featT[:].rearrange("p (k j) -> p k j", k=FT)
            we3 = wenc[:].rearrange("p (k h) -> p k h", k=FT)
            wi3 = wic[:].rearrange("p (k g) -> p k g", k=FT)
            wd3 = wdec[:].rearrange("p (k h) -> p k h", k=HT)
            wh3 = whh[:].rearrange("p (k g) -> p k g", k=HT)

            ftd = featT_d.rearrange("(k p) j -> p k j", p=128)
            wed = wenc_d.rearrange("(k p) h -> p k h", p=128)
            wid = wic_d.rearrange("(k p) g -> p k g", p=128)
            nc.gpsimd.dma_start(ft3[:, 0:4], ftd[:, 0:4])
            nc.sync.dma_start(ft3[:, 4:8], ftd[:, 4:8])
            nc.gpsimd.dma_start(we3[:, 0:4], wed[:, 0:4])
            nc.sync.dma_start(we3[:, 4:8], wed[:, 4:8])
            nc.gpsimd.dma_start(ft3[:, 8:12], ftd[:, 8:12])
            nc.sync.dma_start(ft3[:, 12:16], ftd[:, 12:16])
            nc.gpsimd.dma_start(we3[:, 8:12], wed[:, 8:12])
            nc.sync.dma_start(we3[:, 12:16], wed[:, 12:16])
            nc.scalar.dma_start(wi3[:, 0:4], wid[:, 0:4])
            nc.gpsimd.dma_start(wi3[:, 4:8], wid[:, 4:8])
            nc.sync.dma_start(wi3[:, 8:12], wid[:, 8:12])
            nc.scalar.dma_start(wi3[:, 12:16], wid[:, 12:16])
            nc.scalar.dma_start(wd3, wdec_d.rearrange("(k p) h -> p k h", p=128))
            nc.scalar.dma_start(wh3, whh_d.rearrange("(k p) g -> p k g", p=128))
            nc.gpsimd.dma_start(vatt[:], vatt_d.rearrange("(k p) o -> p (k o)", p=128))
            nc.gpsimd.dma_start(biasT[:], biasT_d.rearrange("(k p) o -> p (k o)", p=128))
            off = 0
            for k in range(4):
                nc.gpsimd.dma_start(mask[k][:], mask_d[off:off + JT[k], :])
                off += JT[k]
            nc.gpsimd.dma_start(maskT[:], maskT_d[:])
            nc.gpsimd.dma_start(ones11[:], ones_d[:])

            ps0cm = tc.tile_pool(name="ps0", bufs=2, space="PSUM")
            ps0 = ps0cm.__enter__()

            # ---- P0a: enc_projT[h, j] = sum_f wenc[f, h] * featT[f, j] + bias
            epT = pp.tile([128, HT * J], dt.bfloat16, tag="epT", name="epT")
            for m in range(HT):
                acc = ps0.tile([128, 512], dt.float32, tag="p0", name="ps_ep")[:, :J]
                for k in range(FT):
                    nc.tensor.matmul(acc[:], we3[:, k, m * 128:(m + 1) * 128],
                                     ft3[:, k, :], start=(k == 0), stop=(k == FT - 1))
                nc.vector.tensor_scalar_add(epT[:, m * J:(m + 1) * J], acc[:],
                                            biasT[:, m:m + 1])

            # ---- P0b: Z[j, g] = sum_f featT[f, j] * wic[f, g]
            Zt = [pp.tile([JT[k], G4], dt.bfloat16, tag=f"Zt{k}", name=f"Zt{k}") for k in range(4)]
            off = 0
            for jm in range(4):
                js = JT[jm]
                for nch in range(4):
                    acc = ps0.tile([128, 512], dt.float32, tag="p0", name="ps_z")[:js, :]
                    for k in range(FT):
                        nc.tensor.matmul(
                            acc[:], ft3[:, k, off:off + js],
                            wi3[:, k, nch * 512:(nch + 1) * 512],
                            start=(k == 0), stop=(k == FT - 1))
                    nc.vector.tensor_copy(Zt[jm][:, nch * 512:(nch + 1) * 512], acc[:])
                off += js

            ps0cm.__exit__(None, None, None)
            p0cm.__exit__(None, None, None)

            # wfc + gathered-h buffers: allocated after the P0 weights free up;
            # the 3.9MB wfc DMA overlaps the recurrence
            latecm = tc.tile_pool(name="late", bufs=1)
            latep = latecm.__enter__()
            mask1 = latep.tile([128, 32], dt.bfloat16, tag="mask1", name="mask1")
            nc.gpsimd.dma_start(mask1[:], mask1_d[:])
            embpT = latep.tile([128, GMT * T * BC], dt.float32, tag="embpT",
                               name="embpT")
            nc.gpsimd.dma_start(embpT[:], embpT_d[:])
            wfcs = latep.tile([128, HT * VS], dt.bfloat16, tag="wfcs", name="wfcs")
            wf3 = wfcs[:].rearrange("p (k v) -> p k v", k=HT)
            nc.gpsimd.dma_start(wf3, wfc_d.rearrange("(k p) v -> p k v", p=128))
            HcAll = latep.tile([128, HT * RALL], dt.bfloat16, tag="HcAll",
                               name="HcAll")

            # ---- recurrence state ----
            cL = pp.tile([128, HT * BC], dt.float32, tag="cL", name="cL")
            Hc = pp.tile([128, T * HT * BC], dt.bfloat16, tag="Hc", name="Hc")  # [h_lo,(t,h_hi,b)]
            nc.gpsimd.memset(cL[:], 0.0)

            Hc4 = Hc[:].rearrange("p (t h b) -> p t h b", t=T, h=HT)

            def hsl(tt, k):   # h(tt) k-tile [128, BC] inside Hc
                return Hc4[:, tt, k, :]

            hc_inA = dram.tile([128, 16 * HT * BC], dt.bfloat16)
            hc_outA = dram.tile([NC * 128, 16 * HT * BC], dt.bfloat16)
            hc_inB = dram.tile([128, 4 * HT * BC], dt.bfloat16)
            hc_outB = dram.tile([NC * 128, 4 * HT * BC], dt.bfloat16)
            # HcAll rows ordered (c, t, b); host reassembles accordingly
            HcA5A = HcAll[:].rearrange("p (k r) -> p k r", k=HT)

            class _HcA5:
                def __getitem__(self, idx):
                    _, k, _, ts, _ = idx
                    if ts == slice(0, 16):
                        return HcA5A[:, k, 0:1024].rearrange(
                            "p (c t b) -> p c t b", c=NC, t=16)
                    assert ts == slice(16, 20)
                    return HcA5A[:, k, 1024:1280].rearrange(
                        "p (c t b) -> p c t b", c=NC, t=4)
            HcA5 = _HcA5()
            HcStage = latep.tile([128, NC * T * HT * BC], dt.bfloat16,
                                 tag="HcStage", name="HcStage")

            pstcm = tc.tile_pool(name="pst", bufs=1, space="PSUM")
            pst = pstcm.__enter__()
            psgcm = tc.tile_pool(name="psg", bufs=1, space="PSUM")
            psg = psgcm.__enter__()

            for t in range(T):
                # 1. decT[h', b] (skip at t=0: h == 0)
                if t > 0:
                    pdec = pst.tile([128, HT * BC], dt.float32, tag="ps_dec", name="ps_dec")
                    for m in range(HT):
                        for k in range(HT):
                            nc.tensor.matmul(
                                pdec[:, m * BC:(m + 1) * BC],
                                wd3[:, k, m * 128:(m + 1) * 128],
                                hsl(t - 1, k),
                                start=(k == 0), stop=(k == HT - 1))
                # Whh-part into its own psum (complete groups) to keep the
                # tensor engine busy during the softmax chain
                if t > 0:
                    pgh = psg.tile([128, GMT * BC], dt.float32, tag="ps_gh",
                                   name="ps_gh")
                    for m in range(GMT):
                        o = m * BC
                        for k in range(HT):
                            nc.tensor.matmul(pgh[:, o:o + BC],
                                             wh3[:, k, m * 128:(m + 1) * 128],
                                             hsl(t - 1, k),
                                             start=(k == 0), stop=(k == HT - 1))
                # 2. R = relu(epT + decT) ; e = v.T @ R
                pe = pst.tile([1, J], dt.float32, tag="ps_e", name="ps_e")
                R = wk.tile([128, HT * J], dt.bfloat16, tag="R", name="R")
                if t > 0:
                    decS = wk.tile([128, HT * BC], dt.bfloat16, tag="decS",
                                   name="decS")
                    nc.vector.tensor_copy(decS[:], pdec[:])
                    radd = wk.tile([128, HT * J], dt.bfloat16, tag="radd",
                                   name="radd")
                    nc.vector.tensor_tensor(
                        radd[:].rearrange("p (m b q) -> p m b q", m=HT, b=BC),
                        epT[:].rearrange("p (m b q) -> p m b q", m=HT, b=BC),
                        decS[:].rearrange("p (m b) -> p m b", m=HT).unsqueeze(3)
                            .broadcast_to([128, HT, BC, P]),
                        ALU.add)
                    nc.vector.tensor_scalar_max(R[:], radd[:], 0.0)
                else:
                    nc.vector.tensor_scalar_max(R[:], epT[:], 0.0)
                for m in range(HT):
                    nc.tensor.matmul(pe[:], vatt[:, m:m + 1], R[:, m * J:(m + 1) * J],
                                     start=(m == 0), stop=(m == HT - 1))
                # 3. softmax over p within each b
                # 3b. s = sigmoid(-e) row; transpose to columns (exact fp32)
                sgn = wk.tile([1, J], dt.float32, tag="sgn", name="sgn")
                nc.scalar.activation(sgn[:], pe[:], AF.Sigmoid, scale=-1.0)
                pa = pst.tile([128, 12], dt.float32, tag="ps_a", name="ps_a")
                off = 0
                for k in range(4):
                    nc.tensor.matmul(pa[:JT[k], k:k + 1],
                                     sgn[:, off:off + JT[k]], ones11[:],
                                     start=True, stop=True)
                    off += JT[k]
                # exp(e) = 1/s - 1, per-column land (128 lanes)
                exr = wk.tile([128, 4], dt.float32, tag="exr", name="exr")
                nc.vector.reciprocal(exr[:], pa[:, 0:4])
                exmb = wk.tile([128, 4], dt.bfloat16, tag="exmb", name="exmb")
                nc.vector.tensor_scalar_add(exmb[:], exr[:], -1.0)
                # per-b sums via mask.T matmul -> pa[0:8, 8]
                for k in range(4):
                    nc.tensor.matmul(pa[0:BC, 8:9], mask[k][:],
                                     exmb[:JT[k], k:k + 1],
                                     start=(k == 0), stop=(k == 3))
                rsc = wk.tile([BC, 1], dt.float32, tag="rsc", name="rsc")
                nc.vector.reciprocal(rsc[:], pa[0:BC, 8:9])
                rscb = wk.tile([BC, 1], dt.bfloat16, tag="rscb", name="rscb")
                nc.vector.tensor_copy(rscb[:], rsc[:])
                # rs392[j] = rs[j//49] via maskT matmul -> pa[:, 4+k]
                off = 0
                for k in range(4):
                    nc.tensor.matmul(pa[:JT[k], 4 + k:5 + k],
                                     maskT[:, off:off + JT[k]], rscb[:],
                                     start=True, stop=True)
                    off += JT[k]
                # 4. A1 = mask1 * exp_col * rs392 (merged [128,(kt,b)], 2 ops)
                At = wk.tile([128, 32], dt.bfloat16, tag="At", name="At")
                nc.vector.tensor_tensor(
                    At[:].rearrange("p (k b) -> p k b", k=4),
                    mask1[:].rearrange("p (k b) -> p k b", k=4),
                    exmb[:].unsqueeze(2).broadcast_to([128, 4, BC]),
                    ALU.mult)
                nc.vector.tensor_tensor(
                    At[:].rearrange("p (k b) -> p k b", k=4),
                    At[:].rearrange("p (k b) -> p k b", k=4),
                    pa[:, 4:8].unsqueeze(2).broadcast_to([128, 4, BC]),
                    ALU.mult)
                # 5. gatesT[g, b] = Z.T @ A
                pg = psg.tile([128, GMT * BC], dt.float32, tag="ps_g", name="ps_g")
                for m in range(GMT):
                    o = m * BC
                    for k in range(4):
                        nc.tensor.matmul(pg[:, o:o + BC],
                                         Zt[k][:, m * 128:(m + 1) * 128],
                                         At[:JT[k], k * BC:(k + 1) * BC],
                                         start=(k == 0),
                                         stop=(k == 3))
                # 6. gates + embp (+ Whh-part), LSTM elementwise
                gL = wk.tile([128, GMT * BC], dt.float32, tag="gL", name="gL")
                nc.vector.tensor_tensor(
                    gL[:].rearrange("p (m b) -> p m b", m=GMT),
                    pg[:].rearrange("p (m b) -> p m b", m=GMT),
                    embpT[:].rearrange("p (m t b) -> p m t b", m=GMT, t=T)[:, :, t, :],
                    ALU.add)
                if t > 0:
                    nc.vector.tensor_tensor(gL[:], gL[:], pgh[:], ALU.add)
                W = HT * BC  # 32 cols per gate; order (i, f, o, g)
                sg = wk.tile([128, 3 * W], dt.float32, tag="sg", name="sg")
                nc.scalar.activation(sg[:], gL[:, 0:3 * W], AF.Sigmoid)
                tg = wk.tile([128, W], dt.float32, tag="tg", name="tg")
                nc.scalar.activation(tg[:], gL[:, 3 * W:4 * W], AF.Tanh)
                si, sf, so = sg[:, 0:W], sg[:, W:2 * W], sg[:, 2 * W:3 * W]
                t1 = wk.tile([128, W], dt.float32, tag="t1", name="t1")
                nc.vector.tensor_tensor(t1[:], sf, cL[:], ALU.mult)
                t2 = wk.tile([128, W], dt.float32, tag="t2", name="t2")
                nc.vector.tensor_tensor(t2[:], si, tg[:], ALU.mult)
                nc.vector.tensor_tensor(cL[:], t1[:], t2[:], ALU.add)
                th = wk.tile([128, W], dt.float32, tag="th", name="th")
                nc.scalar.activation(th[:], cL[:], AF.Tanh)
                nc.vector.tensor_tensor(
                    Hc4[:, t],
                    so.rearrange("p (h b) -> p h b", h=HT),
                    th[:].rearrange("p (h b) -> p h b", h=HT), ALU.mult)
                if t == 15:
                    # gather steps 0..15 now; latency hides under steps 16..19
                    nc.gpsimd.dma_start(hc_inA[:], Hc[:, 0:16 * HT * BC])
                    nc.gpsimd.collective_compute(
                        "AllGather", mybir.AluOpType.bypass,
                        replica_groups=[list(range(NC))],
                        ins=[hc_inA.opt()], outs=[hc_outA.opt()])

            psgcm.__exit__(None, None, None)
            pstcm.__exit__(None, None, None)

            # ---- tail AllGather (steps 16..19); hides under FC on t<16 rows
            nc.gpsimd.dma_start(hc_inB[:], Hc[:, 16 * HT * BC:])
            nc.gpsimd.collective_compute(
                "AllGather", mybir.AluOpType.bypass,
                replica_groups=[list(range(NC))],
                ins=[hc_inB.opt()], outs=[hc_outB.opt()])
            # load-backs on the scalar queue (don't block gpsimd), then
            # on-chip vector reshuffle into HcAll
            hoA = hc_outA[:].rearrange("(c p) r -> p c r", p=128)
            hoB = hc_outB[:].rearrange("(c p) r -> p c r", p=128)
            for c in range(NC):
                nc.scalar.dma_start(
                    HcStage[:, c * 512:(c + 1) * 512], hoA[:, c])
            hsA = HcStage[:, 0:NC * 512].rearrange(
                "p (c t k b) -> p c t k b", c=NC, t=16, k=HT)
            for k in range(HT):
                nc.vector.tensor_copy(HcA5[:, k, :, 0:16, :],
                                      hsA[:, :, :, k, :])
            for c in range(NC):
                nc.scalar.dma_start(
                    HcStage[:, NC * 512 + c * 128:NC * 512 + (c + 1) * 128],
                    hoB[:, c])
            hsB = HcStage[:, NC * 512:].rearrange(
                "p (c t k b) -> p c t k b", c=NC, t=4, k=HT)
            for k in range(HT):
                nc.vector.tensor_copy(HcA5[:, k, :, 16:20, :],
                                      hsB[:, :, :, k, :])

            # ---- FC: out[r, v] = sum_h HcAll[h, r] * wfc[h, v], rows r=(b,t)
            psfcm = tc.tile_pool(name="psfc", bufs=1, space="PSUM")
            psf = psfcm.__enter__()
            focm = tc.tile_pool(name="fcout", bufs=2)
            fco = focm.__enter__()
            for rt in range(NRT):
                pls = [psf.tile([128, 480], dt.float32, tag=f"ps_l{n}",
                                name=f"ps_l{n}") for n in range(NVC)]
                for k in range(HT):
                    st = HcAll[:, k * RALL + rt * 128:k * RALL + (rt + 1) * 128]
                    for n in range(NVC):
                        nc.tensor.matmul(pls[n][:], st,
                                         wf3[:, k, n * 480:(n + 1) * 480],
                                         start=(k == 0), stop=(k == HT - 1))
                outrow = fco.tile([128, VS], dt.bfloat16, tag="outrow",
                                  name="outrow")
                for n in range(NVC):
                    nc.vector.tensor_copy(outrow[:, n * 480:(n + 1) * 480],
                                          pls[n][:])
                    if n == NVC // 2 - 1:
                        nc.gpsimd.dma_start(
                            out_d[rt * 128:(rt + 1) * 128, 0:VS // 2],
                            outrow[:, 0:VS // 2])
                nc.gpsimd.dma_start(out_d[rt * 128:(rt + 1) * 128, VS // 2:],
                                    outrow[:, VS // 2:])
            focm.__exit__(None, None, None)
            psfcm.__exit__(None, None, None)
            latecm.__exit__(None, None, None)
    nc.compile()
    return nc


def _prep_inputs(features, captions, emb_table, W_enc, b_enc, W_dec, b_dec,
                 v_att, b_att, W_ih, b_ih, W_hh, b_hh, W_fc, b_fc):
    f32 = np.float32
    import ml_dtypes
    bf16 = ml_dtypes.bfloat16

    # gate permutation (i, f, g, o) -> (i, f, o, g) on the 4H axis
    gperm = np.concatenate([np.arange(0, H), np.arange(H, 2 * H),
                            np.arange(3 * H, 4 * H), np.arange(2 * H, 3 * H)])
    emb = np.asarray(emb_table, f32)[np.asarray(captions)]        # [B,T,E]
    embp = emb.reshape(B * T, E) @ np.asarray(W_ih, f32)[:E]      # [B*T,4H]
    embp += (np.asarray(b_ih, f32) + np.asarray(b_hh, f32))
    embp = embp.reshape(B, T, G4)[:, :, gperm]

    wicT = np.ascontiguousarray(np.asarray(W_ih, f32)[E:][:, gperm]).astype(bf16)
    wencT = np.asarray(W_enc, f32).astype(bf16)                   # [F,H]
    wdecT = np.asarray(W_dec, f32).astype(bf16)                   # [H,H]
    whhT = np.ascontiguousarray(np.asarray(W_hh, f32)[:, gperm]).astype(bf16)
    vattc = np.asarray(v_att, f32).reshape(H, 1).astype(bf16)
    biasT = (np.asarray(b_enc, f32) + np.asarray(b_dec, f32)).reshape(H, 1)
    wfc = np.zeros((H, VP), f32)
    wfc[:, :V] = np.asarray(W_fc, f32)
    wfc = wfc.astype(bf16)

    maskM = np.zeros((J, BC), f32)
    for j in range(J):
        maskM[j, j // P] = 1.0
    maskM = maskM.astype(bf16)
    maskT = np.ascontiguousarray(maskM.T)
    mask1 = np.zeros((128, 32), f32)
    for k in range(4):
        js = [128, 128, 128, J - 384][k]
        for r in range(js):
            j = k * 128 + r
            mask1[r, k * 8 + j // P] = 1.0
    mask1 = mask1.astype(bf16)
    ones11 = np.ones((1, 1), f32)

    feats = np.asarray(features, f32)
    in_maps = []
    for c in range(NC):
        fs = feats[c * BC:(c + 1) * BC]                           # [8,49,F]
        featT = np.ascontiguousarray(
            fs.reshape(J, F).T).astype(bf16)                      # [F, J]
        ep = embp[c * BC:(c + 1) * BC]                            # [8,T,4H]
        # embpT[g_lo, (mt, t, b)] = ep[b, t, mt*128+g_lo]
        epr = ep.transpose(2, 1, 0).reshape(GMT, 128, T, BC)      # [mt,g_lo,t,b]
        embpT = np.ascontiguousarray(
            epr.transpose(1, 0, 2, 3).reshape(128, GMT * T * BC))
        in_maps.append({
            "featT": featT, "wenc": wencT, "wic": wicT, "wdec": wdecT,
            "whh": whhT, "vatt": vattc, "biasT": biasT.astype(f32),
            "embpT": embpT.astype(f32), "mask": maskM, "maskT": maskT,
            "mask1": mask1, "ones11": ones11,
            "wfc": np.ascontiguousarray(wfc[:, c * VS:(c + 1) * VS]),
        })
    return in_maps


def _install_ntff_hook_shim():
    """Synthesize antenv.axon_hooks (missing in this image) so
    run_bass_kernel_spmd(trace=True) can NTFF-profile via libaxon."""
    import sys, types, ctypes, contextlib
    try:
        from antenv.axon_hooks import get_axon_ntff_profile_hook  # noqa
        return
    except ImportError:
        pass
    so_path = "/opt/axon/libaxon_pjrt.so"
    lib = ctypes.CDLL(so_path)
    lib.axon_start_nrt_profile.argtypes = [ctypes.POINTER(ctypes.c_int64),
                                           ctypes.c_size_t]
    lib.axon_start_nrt_profile.restype = ctypes.c_int64
    lib.axon_stop_nrt_profile.argtypes = [ctypes.c_char_p]
    lib.axon_stop_nrt_profile.restype = ctypes.c_int64

    @contextlib.contextmanager
    def _hook(output_dir, device_ids):
        import jax
        jax.devices()
        if device_ids:
            ids = (ctypes.c_int64 * len(device_ids))(*device_ids)
            rc = lib.axon_start_nrt_profile(ids, len(device_ids))
        else:
            rc = lib.axon_start_nrt_profile(None, 0)
        if rc != 0:
            raise RuntimeError(f"axon_start_nrt_profile rc={rc}")
        try:
            yield
        finally:
            n = lib.axon_stop_nrt_profile(str(output_dir).encode())
            print(f"profile: {n} file(s) written to {output_dir}",
                  file=sys.stderr)

    mod = types.ModuleType("antenv.axon_hooks")
    mod.get_axon_ntff_profile_hook = lambda: _hook
    mod.set_axon_ntff_profile_hook = lambda h: None
    sys.modules["antenv.axon_hooks"] = mod


def kernel(**inputs):
    import os
    from concourse.bass_utils import run_bass_kernel_spmd
    if "nc" not in _cache:
        _cache["nc"] = _build_program()
    nc = _cache["nc"]
    in_maps = _prep_inputs(**inputs)
    trace = bool(int(os.environ.get("KERNEL_TRACE", "0")))
    if trace:
        _install_ntff_hook_shim()
    res = run_bass_kernel_spmd(nc, in_maps, list(range(NC)), trace=trace,
                               tmpdir=os.environ.get("KERNEL_TRACE_DIR"))
    _cache["last_res"] = res
    # each core returns all 1280 (b,t) rows x its 3840-col vocab slice
    def reorder(arr):
        a = arr[:1024].reshape(NC, 16, BC, VS).transpose(0, 2, 1, 3)
        b = arr[1024:].reshape(NC, 4, BC, VS).transpose(0, 2, 1, 3)
        return np.concatenate([a, b], axis=2).reshape(B, T, VS)
    parts = [reorder(np.asarray(res.results[c]["logits"]).astype(np.float32))
             for c in range(NC)]
    out = np.concatenate(parts, axis=2)[:, :, :V]
    out += np.asarray(inputs["b_fc"], np.float32)[None, None, :]
    return out
